# revision 17
# baseline (speedup 1.0000x reference)
"""Distributed Trainium2 kernel for a causal transformer block (pre-LN MHA + FFN).

nn_AttentionThinkingBlock: B=2, S=2048, D=1024, H=16 heads (HS=64), FFN 4096.

Sharding (SPMD-uniform across 8 cores; per-core identity lives in the DATA):
  - Tokens: global 4096 tokens split into 8 contiguous blocks of 512;
    core c owns block c (batch c//4, block c%4). LN1/LN2, proj, FFN and the
    residual stream are computed token-sharded (no all-reduce anywhere).
  - Attention: tensor-parallel over heads. Core c computes Q/K/V and causal
    attention for head pair (2c, 2c+1) over ALL 4096 tokens — identical loop
    structure on every core; which heads = which weight columns the host
    feeds that core.
  - Collectives: AllGather of LN1 output (1 MB/core bf16), AllToAll of the
    attention output (1 MB) to return from head-sharding to token-sharding.

Layout: activations stay transposed ([dim, token]) the whole way so every
matmul contracts over the partition axis; no on-chip transposes except V
(PE-transpose, 32 tiles). Per-token scalars (LN mean/rstd, softmax 1/sum)
are broadcast across partitions with K=1 / K=2 outer-product matmuls.
Softmax row-sums come free from an appended all-ones column on V (M=65 AV
matmul). Compute in bf16, fp32 PSUM accumulation, fp32 residual path.
"""

import sys

sys.path.insert(0, "/opt/trn_rl_repo")

import numpy as np
import ml_dtypes

import concourse.bass as bass
import concourse.tile as tile
from concourse import bacc, mybir
from concourse.bass_utils import run_bass_kernel_spmd
from concourse.masks import make_identity

F32 = mybir.dt.float32
BF16 = mybir.dt.bfloat16
AF = mybir.ActivationFunctionType
ALU = mybir.AluOpType

B, S, D, H, HS = 2, 2048, 1024, 16, 64
EPS = 1e-5
NC = 8
TPC = 512            # tokens per core (residual/FFN shard)
T = B * S            # 4096 global tokens
NEG = -30000.0

BF = ml_dtypes.bfloat16


def _build(use_beta1: bool, use_beta2: bool):
    nc = bacc.Bacc("TRN2", target_bir_lowering=False, debug=False,
                   enable_asserts=False, num_devices=NC)

    def inp(name, shape, dt):
        return nc.dram_tensor(name, shape, dt, kind="ExternalInput")

    xT_e = inp("xT", [D, TPC], F32)          # my token block, transposed
    wqkv_e = inp("wqkv", [D, 384], BF16)     # my 2 heads: q(128)|k(128)|v(128)
    wp_e = inp("wproj", [D, D], BF16)
    w1_e = inp("w1", [D, 4 * D], BF16)
    w2_e = inp("w2", [4 * D, D], BF16)
    consts_e = inp("consts", [4, 128], F32)  # row0 ones
    sel0_e = inp("sel0", [1, 128], F32)      # head0 selector (cols 0-63)
    sel1_e = inp("sel1", [1, 128], F32)      # head1 selector (cols 64-127)
    onesf_e = inp("ones_f32", [128, 1], F32)
    mask_e = inp("mask", [512, 512], BF16)    # causal additive tri mask (0/NEG)
    g1_e = inp("g1", [128, 8], F32)
    g2_e = inp("g2", [128, 8], F32)
    b1_e = inp("b1s", [128, 32], F32)
    bp_e = inp("bprojs", [128, 8], F32)
    b2_e = inp("b2s", [128, 8], F32)
    bt1_e = inp("beta1", [128, 8], F32) if use_beta1 else None
    bt2_e = inp("beta2", [128, 8], F32) if use_beta2 else None
    out_e = nc.dram_tensor("out", [D, TPC], F32, kind="ExternalOutput")

    from contextlib import ExitStack
    with tile.TileContext(nc) as tc:
        with ExitStack() as ctx:
            ep = ctx.enter_context
            dpool = ep(tc.tile_pool(name="dram", bufs=1, space="DRAM"))
            cpool = ep(tc.tile_pool(name="const", bufs=1))
            xpool = ep(tc.tile_pool(name="xT", bufs=1))
            apool = ep(tc.tile_pool(name="actT", bufs=1))     # xnT_mine / hnT
            bpool = ep(tc.tile_pool(name="big", bufs=1))      # xnT_full -> rT
            qkpool = ep(tc.tile_pool(name="qk", bufs=2))      # qT + kT
            mpool = ep(tc.tile_pool(name="mid", bufs=1))      # vT_sb / spare
            vapool = ep(tc.tile_pool(name="vaug", bufs=1))
            opool = ep(tc.tile_pool(name="oT", bufs=1))
            oapool = ep(tc.tile_pool(name="oTall", bufs=1))
            wbpool = ep(tc.tile_pool(name="wbig", bufs=2))    # wproj / w2 slabs
            wspool = ep(tc.tile_pool(name="wslab", bufs=4))   # qkv/w1 col slabs
            hpool = ep(tc.tile_pool(name="hT", bufs=1))
            tpool = ep(tc.tile_pool(name="t1", bufs=2))
            prpool = ep(tc.tile_pool(name="probs", bufs=2))
            rpool = ep(tc.tile_pool(name="rows", bufs=3))
            repool = ep(tc.tile_pool(name="recip", bufs=1))
            bcpool = ep(tc.tile_pool(name="bc", bufs=1))
            outpool = ep(tc.tile_pool(name="outT", bufs=1))
            ps = ep(tc.tile_pool(name="ps", bufs=2, space="PSUM"))
            pa = ep(tc.tile_pool(name="pa", bufs=1, space="PSUM"))
            pov = ep(tc.tile_pool(name="pov", bufs=2, space="PSUM"))
            pl = ep(tc.tile_pool(name="pl", bufs=2, space="PSUM"))
            # ---------------- constants ----------------
            consts = cpool.tile([4, 128], F32)
            nc.sync.dma_start(consts, consts_e[:])
            sel0 = cpool.tile([1, 128], F32)
            nc.sync.dma_start(sel0, sel0_e[:])
            sel1 = cpool.tile([1, 128], F32)
            nc.sync.dma_start(sel1, sel1_e[:])
            onesf = cpool.tile([128, 1], F32)
            nc.sync.dma_start(onesf, onesf_e[:])
            mask_sb = cpool.tile([128, 4, 512], BF16)
            nc.sync.dma_start(mask_sb,
                              mask_e.ap().rearrange("(t p) q -> p t q", p=128))
            g1 = cpool.tile([128, 8], F32)
            nc.sync.dma_start(g1, g1_e[:])
            g2 = cpool.tile([128, 8], F32)
            nc.sync.dma_start(g2, g2_e[:])
            b1s = cpool.tile([128, 32], F32)
            nc.sync.dma_start(b1s, b1_e[:])
            bps = cpool.tile([128, 8], F32)
            nc.sync.dma_start(bps, bp_e[:])
            b2s = cpool.tile([128, 8], F32)
            nc.sync.dma_start(b2s, b2_e[:])
            bt1 = bt2 = None
            if use_beta1:
                bt1 = cpool.tile([128, 8], F32)
                nc.sync.dma_start(bt1, bt1_e[:])
            if use_beta2:
                bt2 = cpool.tile([128, 8], F32)
                nc.sync.dma_start(bt2, bt2_e[:])
            ident = cpool.tile([128, 128], BF16)
            make_identity(nc, ident[:])

            xT = xpool.tile([128, 8, TPC], F32)
            nc.sync.dma_start(xT, xT_e.ap().rearrange("(m p) t -> p m t", p=128))

            # ---------------- layernorm helper (T layout) ----------------
            def layernorm(inT, g_sb, beta_sb, outT):
                """inT [128,8,TPC] f32 -> outT [128,8,TPC] bf16: LN * g + beta."""
                mu_ps = pl.tile([1, TPC], F32, tag="pl")
                sq_ps = pl.tile([1, TPC], F32, tag="pl")
                for m in range(8):
                    nc.tensor.matmul(mu_ps, onesf, inT[:, m, :],
                                     start=(m == 0), stop=(m == 7))
                for m in range(8):
                    sq = tpool.tile([128, TPC], F32, tag="t1")
                    nc.scalar.square(sq, inT[:, m, :])
                    nc.tensor.matmul(sq_ps, onesf, sq,
                                     start=(m == 0), stop=(m == 7))
                mu_row = rpool.tile([1, TPC], F32, tag="rows")
                nc.vector.tensor_scalar_mul(mu_row, mu_ps, 1.0 / D)
                t_row = rpool.tile([1, TPC], F32, tag="rows")
                nc.vector.tensor_scalar_mul(t_row, sq_ps, 1.0 / D)
                mm_row = rpool.tile([1, TPC], F32, tag="rows")
                nc.vector.tensor_mul(mm_row, mu_row, mu_row)
                nc.vector.tensor_sub(t_row, t_row, mm_row)
                nc.vector.tensor_scalar_add(t_row, t_row, EPS)
                nc.scalar.sqrt(t_row, t_row)
                rstd_row = rpool.tile([1, TPC], F32, tag="rows")
                nc.vector.reciprocal(rstd_row, t_row)
                mu_bc = ps.tile([128, TPC], F32, tag="ps")
                nc.tensor.matmul(mu_bc, consts[0:1, :], mu_row)
                rs_bc = ps.tile([128, TPC], F32, tag="ps")
                nc.tensor.matmul(rs_bc, consts[0:1, :], rstd_row)
                bc2 = bcpool.tile([128, 2, TPC], F32, tag="bc")
                nc.scalar.copy(bc2[:, 0, :], mu_bc)
                nc.scalar.copy(bc2[:, 1, :], rs_bc)
                for m in range(8):
                    t1 = tpool.tile([128, TPC], F32, tag="t1")
                    nc.vector.tensor_sub(t1, inT[:, m, :], bc2[:, 0, :])
                    nc.vector.scalar_tensor_tensor(outT[:, m, :], t1,
                                                   g_sb[:, m:m + 1], bc2[:, 1, :],
                                                   op0=ALU.mult, op1=ALU.mult)
                    if beta_sb is not None:
                        nc.vector.tensor_scalar_add(outT[:, m, :], outT[:, m, :],
                                                    beta_sb[:, m:m + 1])

            # ---------------- LN1 + AllGather xn ----------------
            xnT = apool.tile([128, 8, TPC], BF16, tag="actT")
            layernorm(xT, g1, bt1, xnT)

            xn_bnc = dpool.tile([D, TPC], BF16)
            xn_gth = dpool.tile([NC * D, TPC], BF16, addr_space="Shared")
            nc.sync.dma_start(xn_bnc[:].rearrange("(m p) t -> p m t", p=128), xnT)
            nc.gpsimd.collective_compute(
                "AllGather", ALU.bypass,
                ins=[xn_bnc[:].opt()], outs=[xn_gth[:].opt()],
                replica_groups=[list(range(NC))])
            # [128, k(d-tile), r(token block), 512]
            xnF = bpool.tile([128, 8, 8, TPC], BF16, tag="big")
            for r in range(NC):
                nc.sync.dma_start(
                    xnF[:, :, r, :],
                    xn_gth[r * D:(r + 1) * D, :].rearrange("(k p) t -> p k t", p=128))

            # ---------------- QKV for my 2 heads over all tokens ----------------
            qT = qkpool.tile([128, 8, TPC], BF16, tag="qk")   # [2h*64, 4096]
            kT = qkpool.tile([128, 8, TPC], BF16, tag="qk")
            vT = mpool.tile([128, 8, TPC], BF16, tag="mid")
            wqkv_r = wqkv_e.ap().rearrange("(k p) (m j) -> m p k j", p=128, j=128)
            dests = [qT, kT, vT]
            for m in range(3):
                slab = wspool.tile([128, 8, 128], BF16, tag="wslab")
                nc.sync.dma_start(slab, wqkv_r[m])
                for r in range(8):
                    pq = ps.tile([128, TPC], F32, tag="ps")
                    for k in range(8):
                        nc.tensor.matmul(pq, slab[:, k, :], xnF[:, k, r, :],
                                         start=(k == 0), stop=(k == 7))
                    nc.scalar.copy(dests[m][:, r, :], pq)

            # V transpose -> natural [kt, d] with appended ones column (M=65 AV).
            # v_aug[:, t, 0:64]=V h0, col 64 = 1; [:, t, 65:129]=V h1, col 129=1
            v_aug = vapool.tile([128, 32, 130], BF16, tag="vaug")
            nc.vector.memset(v_aug[:, :, 64], 1.0)
            nc.vector.memset(v_aug[:, :, 129], 1.0)
            for tt in range(32):
                pt = pa.tile([128, 128], BF16, tag="pa")
                nc.tensor.transpose(pt, vT[:, tt // 4, (tt % 4) * 128:(tt % 4 + 1) * 128],
                                    ident)
                nc.scalar.copy(v_aug[:, tt, 0:64], pt[:, 0:64])
                nc.scalar.copy(v_aug[:, tt, 65:129], pt[:, 64:128])

            # ---------------- causal attention (my 2 heads, all tokens) --------
            # q groups of 512 tokens per batch; kv tiles of 128.
            oT = opool.tile([128, 8, TPC], BF16, tag="oT")
            for b in range(B):
                for gq in range(4):              # 512-token q group within batch
                    qcol = b * 4 + gq            # block index in qT/kT cols
                    nkv = 4 * gq + 4             # kv tiles of 128
                    po0 = pov.tile([128, TPC], F32, tag="pov")
                    po1 = pov.tile([128, TPC], F32, tag="pov")
                    for t in range(nkv):
                        kb = b * 4 + t // 4      # kv block index
                        kt = (t % 4) * 128
                        pss = pa.tile([128, 2, TPC], F32, tag="pa")
                        nc.tensor.matmul(pss[:, 0, :], kT[0:64, kb, kt:kt + 128],
                                         qT[0:64, qcol, :],
                                         tile_position=(0, 0))
                        nc.tensor.matmul(pss[:, 1, :], kT[64:128, kb, kt:kt + 128],
                                         qT[64:128, qcol, :],
                                         tile_position=(64, 0))
                        diag = t - 4 * gq
                        if diag >= 0:
                            nc.vector.tensor_add(pss[:, 0, :], pss[:, 0, :],
                                                 mask_sb[:, diag, :])
                            nc.vector.tensor_add(pss[:, 1, :], pss[:, 1, :],
                                                 mask_sb[:, diag, :])
                        prb = prpool.tile([128, 2, TPC], BF16, tag="probs")
                        nc.scalar.activation(prb[:, 0, :], pss[:, 0, :], AF.Exp,
                                             scale=0.125)
                        nc.scalar.activation(prb[:, 1, :], pss[:, 1, :], AF.Exp,
                                             scale=0.125)
                        vt = kb * 4 + (t % 4)
                        nc.tensor.matmul(po0[0:65, :], v_aug[:, vt, 0:65],
                                         prb[:, 0, :],
                                         start=(t == 0), stop=(t == nkv - 1))
                        nc.tensor.matmul(po1[0:65, :], v_aug[:, vt, 65:130],
                                         prb[:, 1, :],
                                         start=(t == 0), stop=(t == nkv - 1))
                    # normalize: recip of row-64 sums, broadcast via K=2 matmul
                    rc0 = repool.tile([1, TPC], F32, tag="recip")
                    nc.vector.reciprocal(rc0, po0[64:65, :])
                    rc1 = repool.tile([1, TPC], F32, tag="recip2")
                    nc.vector.reciprocal(rc1, po1[64:65, :])
                    pbc = pa.tile([128, TPC], F32, tag="pa")
                    nc.tensor.matmul(pbc, sel0, rc0, start=True, stop=False)
                    nc.tensor.matmul(pbc, sel1, rc1, start=False, stop=True)
                    bcs = tpool.tile([128, TPC], F32, tag="t1")
                    nc.scalar.copy(bcs, pbc)
                    nc.vector.tensor_mul(oT[0:64, qcol, :], po0[0:64, :],
                                         bcs[0:64, :])
                    nc.vector.tensor_mul(oT[64:128, qcol, :], po1[0:64, :],
                                         bcs[64:128, :])

            # ---------------- AllToAll: head-shard -> token-shard --------------
            o_bnc = dpool.tile([NC, 128, TPC], BF16)
            o_gth = dpool.tile([NC, 128, TPC], BF16)
            nc.sync.dma_start(o_bnc[:].rearrange("j p t -> p j t"), oT)
            nc.gpsimd.collective_compute(
                "AllToAll", ALU.bypass,
                ins=[o_bnc[:].opt()], outs=[o_gth[:].opt()],
                replica_groups=[list(range(NC))])
            oTa = oapool.tile([128, 8, TPC], BF16, tag="oTall")
            nc.sync.dma_start(oTa, o_gth[:].rearrange("j p t -> p j t"))

            # ---------------- proj + residual ----------------
            wp_r = wp_e.ap().rearrange("(k p) (m j) -> m p k j", p=128, j=128)
            hT = hpool.tile([128, 8, TPC], F32, tag="hT")
            for m in range(8):
                wps = wspool.tile([128, 8, 128], BF16, tag="wslab")
                nc.sync.dma_start(wps, wp_r[m])
                pp = ps.tile([128, TPC], F32, tag="ps")
                for k in range(8):
                    nc.tensor.matmul(pp, wps[:, k, :], oTa[:, k, :],
                                     start=(k == 0), stop=(k == 7))
                nc.vector.scalar_tensor_tensor(hT[:, m, :], pp, bps[:, m:m + 1],
                                               xT[:, m, :],
                                               op0=ALU.add, op1=ALU.add)

            # ---------------- LN2 ----------------
            hnT = apool.tile([128, 8, TPC], BF16, tag="actT")
            layernorm(hT, g2, bt2, hnT)

            # ---------------- FFN ----------------
            rT = bpool.tile([128, 32, TPC], BF16, tag="big")
            w1_r = w1_e.ap().rearrange("(k p) (m j) -> m p k j", p=128, j=128)
            for mh in range(32):
                slab = wspool.tile([128, 8, 128], BF16, tag="wslab")
                nc.sync.dma_start(slab, w1_r[mh])
                pr = ps.tile([128, TPC], F32, tag="ps")
                for k in range(8):
                    nc.tensor.matmul(pr, slab[:, k, :], hnT[:, k, :],
                                     start=(k == 0), stop=(k == 7))
                nc.scalar.activation(rT[:, mh, :], pr, AF.Relu,
                                     bias=b1s[:, mh:mh + 1])

            w2_r = w2_e.ap().rearrange("(k p) (m j) -> m p k j", p=128, j=128)
            for m in range(8):
                slab2 = wbpool.tile([128, 32, 128], BF16, tag="wbig")
                nc.sync.dma_start(slab2, w2_r[m])
                pf = ps.tile([128, TPC], F32, tag="ps")
                for k in range(32):
                    nc.tensor.matmul(pf, slab2[:, k, :], rT[:, k, :],
                                     start=(k == 0), stop=(k == 31))
                ot = outpool.tile([128, TPC], F32, tag="outT")
                nc.vector.scalar_tensor_tensor(ot, pf, b2s[:, m:m + 1],
                                               hT[:, m, :],
                                               op0=ALU.add, op1=ALU.add)
                nc.sync.dma_start(out_e[m * 128:(m + 1) * 128, :], ot)

    nc.compile()
    return nc


_CACHE = {}


def _get_nc(use_beta1, use_beta2):
    key = (use_beta1, use_beta2)
    if key not in _CACHE:
        _CACHE[key] = _build(use_beta1, use_beta2)
    return _CACHE[key]


def _prep_inputs(x, wq, wk, wv, wproj, bproj, w1, b1, w2, b2,
                 ln1_g, ln1_b, ln2_g, ln2_b):
    """Build the per-core input maps (all numpy, host side)."""
    x = np.asarray(x, np.float32)
    f32 = lambda a: np.ascontiguousarray(np.asarray(a, np.float32))
    bf16 = lambda a: np.ascontiguousarray(np.asarray(a, np.float32).astype(BF))

    wq, wk, wv = f32(wq), f32(wk), f32(wv)
    # [H, D, HS] -> [D, H*HS] with column order h*64+d
    WQ = np.transpose(wq, (1, 0, 2)).reshape(D, D)
    WK = np.transpose(wk, (1, 0, 2)).reshape(D, D)
    WV = np.transpose(wv, (1, 0, 2)).reshape(D, D)

    consts = np.zeros((4, 128), np.float32)
    consts[0, :] = 1.0
    sel0 = np.zeros((1, 128), np.float32)
    sel0[0, 0:64] = 1.0
    sel1 = np.zeros((1, 128), np.float32)
    sel1[0, 64:128] = 1.0
    onesf = np.ones((128, 1), np.float32)

    mask = np.zeros((512, 512), np.float32)
    kt_i = np.arange(512)[:, None]
    qt_i = np.arange(512)[None, :]
    mask[kt_i > qt_i] = NEG
    mask = mask.astype(BF)

    perpart = lambda v, cols: np.ascontiguousarray(
        np.asarray(v, np.float32).reshape(cols, 128).T)  # [128, cols]

    g1 = perpart(ln1_g, 8)
    g2 = perpart(ln2_g, 8)
    b1s = perpart(b1, 32)
    bps = perpart(bproj, 8)
    b2s = perpart(b2, 8)
    bt1 = perpart(ln1_b, 8)
    bt2 = perpart(ln2_b, 8)
    use_beta1 = bool(np.any(np.asarray(ln1_b)))
    use_beta2 = bool(np.any(np.asarray(ln2_b)))

    xg = x.reshape(T, D)  # global tokens
    in_maps = []
    for c in range(NC):
        hp = slice(2 * c * HS, (2 * c + 2) * HS)  # my two heads' columns
        wqkv = np.concatenate([WQ[:, hp], WK[:, hp], WV[:, hp]], axis=1)
        m = {
            "xT": np.ascontiguousarray(xg[c * TPC:(c + 1) * TPC, :].T),
            "wqkv": bf16(wqkv),
            "wproj": bf16(wproj),
            "w1": bf16(w1),
            "w2": bf16(w2),
            "consts": consts,
            "sel0": sel0,
            "sel1": sel1,
            "ones_f32": onesf,
            "mask": mask,
            "g1": g1, "g2": g2,
            "b1s": b1s, "bprojs": bps, "b2s": b2s,
        }
        if use_beta1:
            m["beta1"] = bt1
        if use_beta2:
            m["beta2"] = bt2
        in_maps.append(m)
    return in_maps, use_beta1, use_beta2


def _run(in_maps, use_beta1, use_beta2, trace=False):
    nc = _get_nc(use_beta1, use_beta2)
    res = run_bass_kernel_spmd(nc, in_maps, list(range(NC)), trace=trace)
    outs = res.results
    out = np.empty((T, D), np.float32)
    for c in range(NC):
        out[c * TPC:(c + 1) * TPC, :] = outs[c]["out"].T
    return out.reshape(B, S, D), res


def kernel(**inputs):
    in_maps, ub1, ub2 = _prep_inputs(**inputs)
    out, _ = _run(in_maps, ub1, ub2, trace=False)
    return out


def _install_ntff_shim():
    """The agent image's antenv lacks axon_hooks; recreate it so
    run_bass_kernel_spmd(trace=True) can capture an NTFF profile."""
    import types
    if "antenv.axon_hooks" in sys.modules:
        return
    try:
        from trn_agent_boot.trn_boot import _ntff_profile_via_ctypes
        hook = _ntff_profile_via_ctypes("/opt/axon/libaxon_pjrt.so")
    except Exception:
        hook = None
    mod = types.ModuleType("antenv.axon_hooks")
    mod._hook = hook
    mod.set_axon_ntff_profile_hook = lambda h: setattr(mod, "_hook", h)
    mod.get_axon_ntff_profile_hook = lambda: mod._hook
    sys.modules["antenv.axon_hooks"] = mod
    import antenv
    antenv.axon_hooks = mod


def kernel_timed(**inputs):
    _install_ntff_shim()
    in_maps, ub1, ub2 = _prep_inputs(**inputs)
    return _run(in_maps, ub1, ub2, trace=True)


# revision 20
# speedup vs baseline: 1.1445x; 1.1445x over previous
"""Distributed Trainium2 kernel for a causal transformer block (pre-LN MHA + FFN).

nn_AttentionThinkingBlock: B=2, S=2048, D=1024, H=16 heads (HS=64), FFN 4096.

Sharding (SPMD-uniform across 8 cores; per-core identity lives in the DATA):
  - Tokens: global 4096 tokens split into 8 contiguous blocks of 512;
    core c owns block c (batch c//4, block c%4). LN1/LN2, proj, FFN and the
    residual stream are computed token-sharded (no all-reduce anywhere).
  - Attention: tensor-parallel over heads. Core c computes Q/K/V and causal
    attention for head pair (2c, 2c+1) over ALL 4096 tokens — identical loop
    structure on every core; which heads = which weight columns the host
    feeds that core.
  - Collectives: AllGather of LN1 output (1 MB/core bf16), AllToAll of the
    attention output (1 MB) to return from head-sharding to token-sharding.

Layout: activations stay transposed ([dim, token]) the whole way so every
matmul contracts over the partition axis; no on-chip transposes except V
(PE-transpose, 32 tiles). Per-token scalars (LN mean/rstd, softmax 1/sum)
are broadcast across partitions with K=1 / K=2 outer-product matmuls.
Softmax row-sums come free from an appended all-ones column on V (M=65 AV
matmul). Compute in bf16, fp32 PSUM accumulation, fp32 residual path.
"""

import sys

sys.path.insert(0, "/opt/trn_rl_repo")

import numpy as np
import ml_dtypes

import concourse.bass as bass
import concourse.tile as tile
from concourse import bacc, mybir
from concourse.bass_utils import run_bass_kernel_spmd
from concourse.masks import make_identity

F32 = mybir.dt.float32
BF16 = mybir.dt.bfloat16
AF = mybir.ActivationFunctionType
ALU = mybir.AluOpType

B, S, D, H, HS = 2, 2048, 1024, 16, 64
EPS = 1e-5
NC = 8
TPC = 512            # tokens per core (residual/FFN shard)
T = B * S            # 4096 global tokens
NEG = -30000.0

BF = ml_dtypes.bfloat16


def _build(use_beta1: bool, use_beta2: bool):
    nc = bacc.Bacc("TRN2", target_bir_lowering=False, debug=False,
                   enable_asserts=False, num_devices=NC)

    def inp(name, shape, dt):
        return nc.dram_tensor(name, shape, dt, kind="ExternalInput")

    xT_e = inp("xT", [D, TPC], F32)          # my token block, transposed
    wqkv_e = inp("wqkv", [D, 384], BF16)     # my 2 heads: q(128)|k(128)|v(128)
    wp_e = inp("wproj", [D, D], BF16)
    w1_e = inp("w1", [D, 4 * D], BF16)
    w2_e = inp("w2", [4 * D, D], BF16)
    consts_e = inp("consts", [4, 128], F32)  # row0 ones
    sel0_e = inp("sel0", [1, 128], F32)      # head0 selector (cols 0-63)
    sel1_e = inp("sel1", [1, 128], F32)      # head1 selector (cols 64-127)
    onesf_e = inp("ones_f32", [128, 1], F32)
    mask_e = inp("mask", [512, 512], BF16)    # causal additive tri mask (0/NEG)
    g1_e = inp("g1", [128, 8], F32)
    g2_e = inp("g2", [128, 8], F32)
    b1_e = inp("b1s", [128, 32], F32)
    bp_e = inp("bprojs", [128, 8], F32)
    b2_e = inp("b2s", [128, 8], F32)
    bt1_e = inp("beta1", [128, 8], F32) if use_beta1 else None
    bt2_e = inp("beta2", [128, 8], F32) if use_beta2 else None
    out_e = nc.dram_tensor("out", [D, TPC], F32, kind="ExternalOutput")

    from contextlib import ExitStack
    with tile.TileContext(nc) as tc:
        with ExitStack() as ctx:
            ep = ctx.enter_context
            dpool = ep(tc.tile_pool(name="dram", bufs=1, space="DRAM"))
            cpool = ep(tc.tile_pool(name="const", bufs=1))
            xpool = ep(tc.tile_pool(name="xT", bufs=1))
            apool = ep(tc.tile_pool(name="actT", bufs=1))     # xnT_mine / hnT
            bpool = ep(tc.tile_pool(name="big", bufs=1))      # xnT_full -> rT
            qkpool = ep(tc.tile_pool(name="qk", bufs=2))      # qT + kT
            mpool = ep(tc.tile_pool(name="mid", bufs=1))      # vT_sb / spare
            vapool = ep(tc.tile_pool(name="vaug", bufs=1))
            opool = ep(tc.tile_pool(name="oT", bufs=1))
            oapool = ep(tc.tile_pool(name="oTall", bufs=1))
            wbpool = ep(tc.tile_pool(name="wbig", bufs=2))    # wproj / w2 slabs
            wspool = ep(tc.tile_pool(name="wslab", bufs=4))   # qkv/w1 col slabs
            hpool = ep(tc.tile_pool(name="hT", bufs=1))
            tpool = ep(tc.tile_pool(name="t1", bufs=2))
            prpool = ep(tc.tile_pool(name="probs", bufs=2))
            rpool = ep(tc.tile_pool(name="rows", bufs=3))
            repool = ep(tc.tile_pool(name="recip", bufs=1))
            bcpool = ep(tc.tile_pool(name="bc", bufs=1))
            outpool = ep(tc.tile_pool(name="outT", bufs=1))
            ps = ep(tc.tile_pool(name="ps", bufs=2, space="PSUM"))
            pov = ep(tc.tile_pool(name="pov", bufs=2, space="PSUM"))
            pl = ep(tc.tile_pool(name="pl", bufs=2, space="PSUM"))
            # ---------------- constants ----------------
            consts = cpool.tile([4, 128], F32)
            nc.sync.dma_start(consts, consts_e[:])
            sel0 = cpool.tile([1, 128], F32)
            nc.sync.dma_start(sel0, sel0_e[:])
            sel1 = cpool.tile([1, 128], F32)
            nc.sync.dma_start(sel1, sel1_e[:])
            onesf = cpool.tile([128, 1], F32)
            nc.sync.dma_start(onesf, onesf_e[:])
            mask_sb = cpool.tile([128, 4, 512], BF16)
            nc.sync.dma_start(mask_sb,
                              mask_e.ap().rearrange("(t p) q -> p t q", p=128))
            g1 = cpool.tile([128, 8], F32)
            nc.sync.dma_start(g1, g1_e[:])
            g2 = cpool.tile([128, 8], F32)
            nc.sync.dma_start(g2, g2_e[:])
            b1s = cpool.tile([128, 32], F32)
            nc.sync.dma_start(b1s, b1_e[:])
            bps = cpool.tile([128, 8], F32)
            nc.sync.dma_start(bps, bp_e[:])
            b2s = cpool.tile([128, 8], F32)
            nc.sync.dma_start(b2s, b2_e[:])
            bt1 = bt2 = None
            if use_beta1:
                bt1 = cpool.tile([128, 8], F32)
                nc.sync.dma_start(bt1, bt1_e[:])
            if use_beta2:
                bt2 = cpool.tile([128, 8], F32)
                nc.sync.dma_start(bt2, bt2_e[:])
            ident = cpool.tile([128, 128], BF16)
            make_identity(nc, ident[:])

            xT = xpool.tile([128, 8, TPC], F32)
            nc.sync.dma_start(xT, xT_e.ap().rearrange("(m p) t -> p m t", p=128))

            # ---------------- layernorm helper (T layout) ----------------
            def layernorm(inT, g_sb, beta_sb, outT):
                """inT [128,8,TPC] f32 -> outT [128,8,TPC] bf16: LN * g + beta."""
                mu_ps = pl.tile([1, TPC], F32, tag="pl")
                sq_ps = pl.tile([1, TPC], F32, tag="pl")
                for m in range(8):
                    nc.tensor.matmul(mu_ps, onesf, inT[:, m, :],
                                     start=(m == 0), stop=(m == 7))
                for m in range(8):
                    sq = tpool.tile([128, TPC], F32, tag="t1")
                    nc.scalar.square(sq, inT[:, m, :])
                    nc.tensor.matmul(sq_ps, onesf, sq,
                                     start=(m == 0), stop=(m == 7))
                mu_row = rpool.tile([1, TPC], F32, tag="rows")
                nc.vector.tensor_scalar_mul(mu_row, mu_ps, 1.0 / D)
                t_row = rpool.tile([1, TPC], F32, tag="rows")
                nc.vector.tensor_scalar_mul(t_row, sq_ps, 1.0 / D)
                mm_row = rpool.tile([1, TPC], F32, tag="rows")
                nc.vector.tensor_mul(mm_row, mu_row, mu_row)
                nc.vector.tensor_sub(t_row, t_row, mm_row)
                nc.vector.tensor_scalar_add(t_row, t_row, EPS)
                nc.scalar.activation(t_row, t_row, AF.Ln)
                nc.scalar.activation(t_row, t_row, AF.Exp, scale=-0.5)
                rstd_row = t_row
                mu_bc = ps.tile([128, TPC], F32, tag="ps")
                nc.tensor.matmul(mu_bc, consts[0:1, :], mu_row)
                rs_bc = ps.tile([128, TPC], F32, tag="ps")
                nc.tensor.matmul(rs_bc, consts[0:1, :], rstd_row)
                bc2 = bcpool.tile([128, 2, TPC], F32, tag="bc")
                nc.scalar.copy(bc2[:, 0, :], mu_bc)
                nc.scalar.copy(bc2[:, 1, :], rs_bc)
                for m in range(8):
                    t1 = tpool.tile([128, TPC], F32, tag="t1")
                    nc.vector.tensor_sub(t1, inT[:, m, :], bc2[:, 0, :])
                    nc.vector.scalar_tensor_tensor(outT[:, m, :], t1,
                                                   g_sb[:, m:m + 1], bc2[:, 1, :],
                                                   op0=ALU.mult, op1=ALU.mult)
                    if beta_sb is not None:
                        nc.vector.tensor_scalar_add(outT[:, m, :], outT[:, m, :],
                                                    beta_sb[:, m:m + 1])

            # ---------------- LN1 + AllGather xn ----------------
            xnT = apool.tile([128, 8, TPC], BF16, tag="actT")
            layernorm(xT, g1, bt1, xnT)

            xn_bnc = dpool.tile([D, TPC], BF16)
            xn_gth = dpool.tile([NC * D, TPC], BF16, addr_space="Shared")
            nc.sync.dma_start(xn_bnc[:].rearrange("(m p) t -> p m t", p=128), xnT)
            nc.gpsimd.collective_compute(
                "AllGather", ALU.bypass,
                ins=[xn_bnc[:].opt()], outs=[xn_gth[:].opt()],
                replica_groups=[list(range(NC))])
            # [128, k(d-tile), r(token block), 512]
            xnF = bpool.tile([128, 8, 8, TPC], BF16, tag="big")
            for r in range(NC):
                nc.sync.dma_start(
                    xnF[:, :, r, :],
                    xn_gth[r * D:(r + 1) * D, :].rearrange("(k p) t -> p k t", p=128))

            # ---------------- QKV for my 2 heads over all tokens ----------------
            qT = qkpool.tile([128, 8, TPC], BF16, tag="qk")   # [2h*64, 4096]
            kT = qkpool.tile([128, 8, TPC], BF16, tag="qk")
            vT = mpool.tile([128, 8, TPC], BF16, tag="mid")
            wqkv_r = wqkv_e.ap().rearrange("(k p) (m j) -> m p k j", p=128, j=128)
            dests = [qT, kT, vT]
            for m in range(3):
                slab = wspool.tile([128, 8, 128], BF16, tag="wslab")
                nc.sync.dma_start(slab, wqkv_r[m])
                for r in range(8):
                    pq = ps.tile([128, TPC], F32, tag="ps")
                    for k in range(8):
                        nc.tensor.matmul(pq, slab[:, k, :], xnF[:, k, r, :],
                                         start=(k == 0), stop=(k == 7))
                    nc.scalar.copy(dests[m][:, r, :], pq)

            # V transpose -> natural [kt, d] with appended ones column (M=65 AV).
            # v_aug[:, t, 0:64]=V h0, col 64 = 1; [:, t, 65:129]=V h1, col 129=1
            v_aug = vapool.tile([128, 32, 130], BF16, tag="vaug")
            nc.vector.memset(v_aug[:, :, 64], 1.0)
            nc.vector.memset(v_aug[:, :, 129], 1.0)
            for tt in range(32):
                pt = pl.tile([128, 128], BF16, tag="pl")
                nc.tensor.transpose(pt, vT[:, tt // 4, (tt % 4) * 128:(tt % 4 + 1) * 128],
                                    ident)
                nc.scalar.copy(v_aug[:, tt, 0:64], pt[:, 0:64])
                nc.scalar.copy(v_aug[:, tt, 65:129], pt[:, 64:128])

            # ---------------- causal attention (my 2 heads, all tokens) --------
            # q groups of 512 tokens per batch; kv tiles of 128.
            oT = opool.tile([128, 8, TPC], BF16, tag="oT")
            for b in range(B):
                for gq in range(4):              # 512-token q group within batch
                    qcol = b * 4 + gq            # block index in qT/kT cols
                    nkv = 4 * gq + 4             # kv tiles of 128
                    po0 = pov.tile([128, TPC], F32, tag="pov")
                    po1 = pov.tile([128, TPC], F32, tag="pov")
                    for t in range(nkv):
                        kb = b * 4 + t // 4      # kv block index
                        kt = (t % 4) * 128
                        pss = pl.tile([128, 2, TPC], F32, tag="pl")
                        nc.tensor.matmul(pss[:, 0, :], kT[0:64, kb, kt:kt + 128],
                                         qT[0:64, qcol, :],
                                         tile_position=(0, 0))
                        nc.tensor.matmul(pss[:, 1, :], kT[64:128, kb, kt:kt + 128],
                                         qT[64:128, qcol, :],
                                         tile_position=(64, 0))
                        diag = t - 4 * gq
                        if diag >= 0:
                            nc.vector.tensor_add(pss[:, 0, :], pss[:, 0, :],
                                                 mask_sb[:, diag, :])
                            nc.vector.tensor_add(pss[:, 1, :], pss[:, 1, :],
                                                 mask_sb[:, diag, :])
                        prb = prpool.tile([128, 2, TPC], BF16, tag="probs")
                        nc.scalar.activation(prb[:, 0, :], pss[:, 0, :], AF.Exp,
                                             scale=0.125)
                        nc.scalar.activation(prb[:, 1, :], pss[:, 1, :], AF.Exp,
                                             scale=0.125)
                        vt = kb * 4 + (t % 4)
                        nc.tensor.matmul(po0[0:65, :], v_aug[:, vt, 0:65],
                                         prb[:, 0, :],
                                         start=(t == 0), stop=(t == nkv - 1))
                        nc.tensor.matmul(po1[0:65, :], v_aug[:, vt, 65:130],
                                         prb[:, 1, :],
                                         start=(t == 0), stop=(t == nkv - 1))
                    # normalize: recip of row-64 sums, broadcast via K=2 matmul
                    rc0 = repool.tile([1, TPC], F32, tag="recip")
                    nc.scalar.activation(rc0, po0[64:65, :], AF.Ln)
                    nc.scalar.activation(rc0, rc0, AF.Exp, scale=-1.0)
                    rc1 = repool.tile([1, TPC], F32, tag="recip2")
                    nc.scalar.activation(rc1, po1[64:65, :], AF.Ln)
                    nc.scalar.activation(rc1, rc1, AF.Exp, scale=-1.0)
                    pbc = pl.tile([128, TPC], F32, tag="pl")
                    nc.tensor.matmul(pbc, sel0, rc0, start=True, stop=False)
                    nc.tensor.matmul(pbc, sel1, rc1, start=False, stop=True)
                    bcs = tpool.tile([128, TPC], F32, tag="t1")
                    nc.scalar.copy(bcs, pbc)
                    nc.vector.tensor_mul(oT[0:64, qcol, :], po0[0:64, :],
                                           bcs[0:64, :])
                    nc.vector.tensor_mul(oT[64:128, qcol, :], po1[0:64, :],
                                           bcs[64:128, :])

            # ---------------- AllToAll: head-shard -> token-shard --------------
            o_bnc = dpool.tile([NC, 128, TPC], BF16)
            o_gth = dpool.tile([NC, 128, TPC], BF16)
            nc.sync.dma_start(o_bnc[:].rearrange("j p t -> p j t"), oT)
            nc.gpsimd.collective_compute(
                "AllToAll", ALU.bypass,
                ins=[o_bnc[:].opt()], outs=[o_gth[:].opt()],
                replica_groups=[list(range(NC))])
            oTa = oapool.tile([128, 8, TPC], BF16, tag="oTall")
            nc.sync.dma_start(oTa, o_gth[:].rearrange("j p t -> p j t"))

            # ---------------- proj + residual ----------------
            wp_r = wp_e.ap().rearrange("(k p) (m j) -> m p k j", p=128, j=128)
            hT = hpool.tile([128, 8, TPC], F32, tag="hT")
            for m in range(8):
                wps = wspool.tile([128, 8, 128], BF16, tag="wslab")
                nc.sync.dma_start(wps, wp_r[m])
                pp = ps.tile([128, TPC], F32, tag="ps")
                for k in range(8):
                    nc.tensor.matmul(pp, wps[:, k, :], oTa[:, k, :],
                                     start=(k == 0), stop=(k == 7))
                nc.vector.scalar_tensor_tensor(hT[:, m, :], pp, bps[:, m:m + 1],
                                               xT[:, m, :],
                                               op0=ALU.add, op1=ALU.add)

            # ---------------- LN2 ----------------
            hnT = apool.tile([128, 8, TPC], BF16, tag="actT")
            layernorm(hT, g2, bt2, hnT)

            # ---------------- FFN ----------------
            rT = bpool.tile([128, 32, TPC], BF16, tag="big")
            w1_r = w1_e.ap().rearrange("(k p) (m j) -> m p k j", p=128, j=128)
            for mh in range(32):
                slab = wspool.tile([128, 8, 128], BF16, tag="wslab")
                nc.sync.dma_start(slab, w1_r[mh])
                pr = ps.tile([128, TPC], F32, tag="ps")
                for k in range(8):
                    nc.tensor.matmul(pr, slab[:, k, :], hnT[:, k, :],
                                     start=(k == 0), stop=(k == 7))
                nc.scalar.activation(rT[:, mh, :], pr, AF.Relu,
                                     bias=b1s[:, mh:mh + 1])

            w2_r = w2_e.ap().rearrange("(k p) (m j) -> m p k j", p=128, j=128)
            for m in range(8):
                slab2 = wbpool.tile([128, 32, 128], BF16, tag="wbig")
                nc.sync.dma_start(slab2, w2_r[m])
                pf = ps.tile([128, TPC], F32, tag="ps")
                for k in range(32):
                    nc.tensor.matmul(pf, slab2[:, k, :], rT[:, k, :],
                                     start=(k == 0), stop=(k == 31))
                ot = outpool.tile([128, TPC], F32, tag="outT")
                nc.vector.scalar_tensor_tensor(ot, pf, b2s[:, m:m + 1],
                                               hT[:, m, :],
                                               op0=ALU.add, op1=ALU.add)
                nc.sync.dma_start(out_e[m * 128:(m + 1) * 128, :], ot)

    nc.compile()
    return nc


_CACHE = {}


def _get_nc(use_beta1, use_beta2):
    key = (use_beta1, use_beta2)
    if key not in _CACHE:
        _CACHE[key] = _build(use_beta1, use_beta2)
    return _CACHE[key]


def _prep_inputs(x, wq, wk, wv, wproj, bproj, w1, b1, w2, b2,
                 ln1_g, ln1_b, ln2_g, ln2_b):
    """Build the per-core input maps (all numpy, host side)."""
    x = np.asarray(x, np.float32)
    f32 = lambda a: np.ascontiguousarray(np.asarray(a, np.float32))
    bf16 = lambda a: np.ascontiguousarray(np.asarray(a, np.float32).astype(BF))

    wq, wk, wv = f32(wq), f32(wk), f32(wv)
    # [H, D, HS] -> [D, H*HS] with column order h*64+d
    WQ = np.transpose(wq, (1, 0, 2)).reshape(D, D)
    WK = np.transpose(wk, (1, 0, 2)).reshape(D, D)
    WV = np.transpose(wv, (1, 0, 2)).reshape(D, D)

    consts = np.zeros((4, 128), np.float32)
    consts[0, :] = 1.0
    sel0 = np.zeros((1, 128), np.float32)
    sel0[0, 0:64] = 1.0
    sel1 = np.zeros((1, 128), np.float32)
    sel1[0, 64:128] = 1.0
    onesf = np.ones((128, 1), np.float32)

    mask = np.zeros((512, 512), np.float32)
    kt_i = np.arange(512)[:, None]
    qt_i = np.arange(512)[None, :]
    mask[kt_i > qt_i] = NEG
    mask = mask.astype(BF)

    perpart = lambda v, cols: np.ascontiguousarray(
        np.asarray(v, np.float32).reshape(cols, 128).T)  # [128, cols]

    g1 = perpart(ln1_g, 8)
    g2 = perpart(ln2_g, 8)
    b1s = perpart(b1, 32)
    bps = perpart(bproj, 8)
    b2s = perpart(b2, 8)
    bt1 = perpart(ln1_b, 8)
    bt2 = perpart(ln2_b, 8)
    use_beta1 = bool(np.any(np.asarray(ln1_b)))
    use_beta2 = bool(np.any(np.asarray(ln2_b)))

    xg = x.reshape(T, D)  # global tokens
    in_maps = []
    for c in range(NC):
        hp = slice(2 * c * HS, (2 * c + 2) * HS)  # my two heads' columns
        wqkv = np.concatenate([WQ[:, hp], WK[:, hp], WV[:, hp]], axis=1)
        m = {
            "xT": np.ascontiguousarray(xg[c * TPC:(c + 1) * TPC, :].T),
            "wqkv": bf16(wqkv),
            "wproj": bf16(wproj),
            "w1": bf16(w1),
            "w2": bf16(w2),
            "consts": consts,
            "sel0": sel0,
            "sel1": sel1,
            "ones_f32": onesf,
            "mask": mask,
            "g1": g1, "g2": g2,
            "b1s": b1s, "bprojs": bps, "b2s": b2s,
        }
        if use_beta1:
            m["beta1"] = bt1
        if use_beta2:
            m["beta2"] = bt2
        in_maps.append(m)
    return in_maps, use_beta1, use_beta2


def _run(in_maps, use_beta1, use_beta2, trace=False):
    nc = _get_nc(use_beta1, use_beta2)
    res = run_bass_kernel_spmd(nc, in_maps, list(range(NC)), trace=trace)
    outs = res.results
    out = np.empty((T, D), np.float32)
    for c in range(NC):
        out[c * TPC:(c + 1) * TPC, :] = outs[c]["out"].T
    return out.reshape(B, S, D), res


def kernel(**inputs):
    in_maps, ub1, ub2 = _prep_inputs(**inputs)
    out, _ = _run(in_maps, ub1, ub2, trace=False)
    return out


def _install_ntff_shim():
    """The agent image's antenv lacks axon_hooks; recreate it so
    run_bass_kernel_spmd(trace=True) can capture an NTFF profile."""
    import types
    if "antenv.axon_hooks" in sys.modules:
        return
    try:
        from trn_agent_boot.trn_boot import _ntff_profile_via_ctypes
        hook = _ntff_profile_via_ctypes("/opt/axon/libaxon_pjrt.so")
    except Exception:
        hook = None
    mod = types.ModuleType("antenv.axon_hooks")
    mod._hook = hook
    mod.set_axon_ntff_profile_hook = lambda h: setattr(mod, "_hook", h)
    mod.get_axon_ntff_profile_hook = lambda: mod._hook
    sys.modules["antenv.axon_hooks"] = mod
    import antenv
    antenv.axon_hooks = mod


def kernel_timed(**inputs):
    _install_ntff_shim()
    in_maps, ub1, ub2 = _prep_inputs(**inputs)
    return _run(in_maps, ub1, ub2, trace=True)
